# revision 24
# baseline (speedup 1.0000x reference)
"""Distributed multi-head attention kernel for 8 TRN2 NeuronCores.

Problem: x[2,2048,1024] -> qkv proj (w_qkv[3072,1024]) -> 16-head SDPA ->
out proj (w_proj[1024,1024], b_proj[1024]).

Sharding: tensor-parallel over heads. Core c owns heads {2c, 2c+1}:
  - stage 1 (per core): q/k/v for its 2 heads over ALL 4096 tokens,
    transposed score tiles S^T[m,n] per (batch, head), exp on the scalar
    engine (no max-subtraction: scores ~ N(0,1), fp32 exp is safe), PV with
    a trailing ones-column in V so PSUM row DH accumulates the softmax
    denominator, then normalize. Result: aT_h [64 head-dims, 4096 tokens].
  - After EVERY attention n-chunk (512 tokens) a small AllToAll reshards
    that quarter from head-parallel to token-parallel: core c receives the
    full 1024 C-dims for its 64-token piece of the quarter.
  - stage 2 (per core): y^T strips = w_proj @ a + b_proj, emitted as soon
    as both quarters of a 128-col strip have arrived, interleaved into the
    later attention chunks. Only a 64-col strip trails the last AllToAll.

Engine-ring discipline (each engine's queue is strict FIFO):
  - ACT (scalar) ring carries ONLY the exp activations — the softmax pace
    is never blocked behind a DMA trigger.
  - sync (SP) ring: bulk loads (weights, x chunks — one batched DMA per
    chunk), denominator moves, reshard a2a writes + gathers, output stores.
  - gpsimd (Pool) ring: partition broadcasts and the AllToAll collectives.
  - DVE: PSUM->SBUF copies, reciprocal, normalize muls, bias adds.
PSUM pools are split by consumer so the score-tile rotation (ACT-consumed)
is never interleaved with DVE-consumed tiles: a filler allocation between
two s_ps allocations would otherwise couple QK^T emission to exp progress.

Stage-1 compute is interleaved into attention chunks as ~430ns units (2
k/q c-tiles, one v m-tile, or one proj output tile per attention m-tile
slot) so every chunk carries at least the ACT exp pace (~16.6us) without
lumps that head-of-line-block the PE queue: k/v of chunk t+1.. forced
early (QK^T and PV consume them), q deferred to the chunk that first needs
it, proj strips fill the late slots of the chunk after their quarter's
reshard lands.

All TensorE matmuls run in bf16 (fp32 PSUM accumulation); softmax exp is
computed in fp32 on the scalar engine straight out of PSUM.
"""

import numpy as np
import ml_dtypes

import concourse.bass as bass
import concourse.bacc as bacc
import concourse.tile as tile
import concourse.mybir as mybir
from concourse import bass_utils

BF16 = mybir.dt.bfloat16
F32 = mybir.dt.float32

N_CORES = 8
B = 2
N = 2048
DIM = 1024
H = 16
DH = 64
SCALE = DH ** -0.5
HPC = H // N_CORES          # heads per core = 2
T = B * N                   # 4096 global tokens
TPC = T // N_CORES          # 512 tokens per core in stage 2
QP = 64                     # tokens per shard piece per quarter-reshard
CT = DIM // 128             # 8 contraction tiles
TCH = 512                   # token chunk for stage-1 matmul streaming
NCH = 512                   # n (query) chunk in attention
MT = N // 128               # 16 m-tiles per batch
NQ = B * 4                  # 8 quarter-reshards (one per n-chunk)

_cached = None


class _Ctx:
    pass


def _load_chunk0(c):
    """Chunk 0 split per c-tile so the first matmul starts after 1/8 of
    the transfer; c-tiles 4-7 go out on the DVE ring so the two DGE queues
    deliver the chunk in parallel (halves the serial trigger time that
    gates the very first QK^T)."""
    xc = c.xin.tile([128, CT, TCH], BF16, tag="xc", name="xc")
    for a in range(CT):
        eng = c.nc.sync if a < 4 else c.nc.vector
        eng.dma_start(xc[:, a, :], c.xT_d[128 * a:128 * (a + 1), 0:TCH])
    return xc


def _load_chunk(c, tci):
    """One batched DMA per token chunk (single HWDGE trigger)."""
    t0 = tci * TCH
    xc = c.xin.tile([128, CT, TCH], BF16, tag="xc", name="xc")
    c.nc.sync.dma_start(
        xc[:], c.xT_d.ap().rearrange("(a p) t -> p a t", p=128)[:, :, t0:t0 + TCH])
    return xc


def _proj_units(c, w_sb, dst, tci):
    """q/k projection of chunk tci as 4 ~430ns units (2 c-tiles each) so the
    filler can spread across attention m-tile slots without starving the
    ACT-paced exp stream."""
    t0 = tci * TCH
    st = {}

    def u(i):
        def f():
            xc = c.xcs[tci]
            if i == 0:
                st["ps"] = c.psA.tile([128, TCH], F32, tag="psA", name="kq_ps")
            ps = st["ps"]
            for a in (2 * i, 2 * i + 1):
                c.nc.tensor.matmul(ps[:], w_sb[:, a, :], xc[:, a, :],
                                   start=(a == 0), stop=(a == CT - 1))
            if i == 3:
                c.nc.vector.tensor_copy(dst[:, t0:t0 + TCH], ps[:])
        return f
    return [u(i) for i in range(4)]


def _k_units(c, tci):
    return _proj_units(c, c.wk_sb, c.kT, tci)


def _q_units(c, tci):
    return _proj_units(c, c.wq_sb, c.qT, tci)


def _v_units(c, tci):
    """v of chunk tci as 4 units (one 128-token m-tile each)."""
    t0 = tci * TCH

    def u(mt):
        def f():
            xc = c.xcs[tci]
            gmt = (t0 // 128) + mt
            v_ps = c.psA.tile([128, 128], F32, tag="psA", name="v_ps")
            for a in range(CT):
                c.nc.tensor.matmul(
                    v_ps[:], xc[:, a, 128 * mt:128 * (mt + 1)], c.wv_sb[:, a, :],
                    start=(a == 0), stop=(a == CT - 1))
            for h in range(HPC):
                c.nc.vector.tensor_copy(
                    c.v_aug[h][:, gmt, 0:DH], v_ps[:, DH * h:DH * (h + 1)])
        return f
    return [u(mt) for mt in range(TCH // 128)]


def _attn_qk_exp(c, b, nci, mt):
    nc = c.nc
    n0 = b * N + nci * NCH
    m0 = b * N + 128 * mt
    s_ps = c.psS.tile([128, HPC * NCH], F32, tag="psS", name="s_ps")
    e_t = c.etp.tile([128, HPC * NCH], BF16, tag="et", name="e_t")
    for h in range(HPC):
        nc.tensor.matmul(
            s_ps[:, NCH * h:NCH * (h + 1)],
            c.kT[DH * h:DH * (h + 1), m0:m0 + 128],
            c.qT[DH * h:DH * (h + 1), n0:n0 + NCH],
            start=True, stop=True,
            tile_position=(DH * h, 0))
    nc.scalar.activation(e_t[:], s_ps[:],
                         mybir.ActivationFunctionType.Exp, scale=SCALE)
    return e_t


def _attn_pv(c, mt, gmt, e_t, o_ps):
    nc = c.nc
    for h in range(HPC):
        nc.tensor.matmul(
            o_ps[h][:], c.v_aug[h][:, gmt, :],
            e_t[:, NCH * h:NCH * (h + 1)],
            start=(mt == 0), stop=(mt == MT - 1))


def _attn_finish(c, b, nci, o_ps):
    """Normalize + quarter-reshard, pipelined per head to minimize the
    serial latency from the last PV to the AllToAll gather (this chain
    gates both the chunk-boundary o_ps buffer turnaround and the kernel
    tail). Row DH of o_ps is the softmax denominator: DVE reciprocal, a
    tiny SP-ring DMA moves it to partition 0, gpsimd broadcasts, DVE
    scales, SP-ring DMA stages the a2a slabs."""
    nc = c.nc
    qi = b * 4 + nci
    n0 = b * N + nci * NCH
    den = c.small.tile([DH + 1, HPC * NCH], F32, tag="den", name="den")
    rden = c.small.tile([1, HPC * NCH], F32, tag="rden", name="rden")
    rb = c.small.tile([DH, HPC * NCH], F32, tag="rb", name="rb")
    for h in range(HPC):
        nc.vector.reciprocal(den[DH:DH + 1, NCH * h:NCH * (h + 1)],
                             o_ps[h][DH:DH + 1, :])
        nc.sync.dma_start(rden[:, NCH * h:NCH * (h + 1)],
                          den[DH:DH + 1, NCH * h:NCH * (h + 1)])
    for h in range(HPC):
        nc.gpsimd.partition_broadcast(rb[:, NCH * h:NCH * (h + 1)],
                                      rden[:, NCH * h:NCH * (h + 1)])
        nc.vector.tensor_mul(c.aT[h][:, n0:n0 + NCH],
                             o_ps[h][0:DH, :],
                             rb[:, NCH * h:NCH * (h + 1)])
        nc.sync.dma_start(
            c.a2a_in[qi][:, h, :, :].rearrange("j d t -> d j t"),
            c.aT[h][:, n0:n0 + NCH])
    if c.use_collective:
        nc.gpsimd.collective_compute(
            "AllToAll", mybir.AluOpType.bypass,
            replica_groups=[list(range(N_CORES))],
            ins=[c.a2a_in[qi].opt()], outs=[c.a2a_out[qi].opt()])
    else:
        nc.gpsimd.dma_start(c.a2a_out[qi][:], c.a2a_in[qi][:])
    nc.sync.dma_start(
        c.agT[:, :, QP * qi:QP * qi + QP],
        c.a2a_out[qi][:].rearrange("w h d t -> (h d) w t"))


def _attn_nchunk(c, b, nci, interleave=None):
    """One attention n-chunk; interleave maps m-tile slot -> list of filler
    callbacks emitted after that m-tile's QK^T/exp + the pipelined PV, to
    fill TensorEngine slack at sub-exp granularity. Ends with this
    quarter's normalize + reshard."""
    o_ps = [c.psB.tile([DH + 1, NCH], F32, tag="psB", name=f"o_ps{h}")
            for h in range(HPC)]
    # software-pipelined: PV of m-tile mt-1 is emitted after QK^T/exp of
    # m-tile mt, so the next QK^T never queues behind an exp-blocked PV.
    prev = None
    for mt in range(MT):
        e_t = _attn_qk_exp(c, b, nci, mt)
        # fillers sit BETWEEN this QK^T and the exp-gated PV of the
        # previous m-tile, so the PE chews filler during the exp wait
        # instead of idling at the PV head-of-line.
        if interleave and mt in interleave:
            for fn in interleave[mt]:
                fn()
        if prev is not None:
            _attn_pv(c, prev[0], b * MT + prev[0], prev[1], o_ps)
        prev = (mt, e_t)
    _attn_pv(c, prev[0], b * MT + prev[0], prev[1], o_ps)
    _attn_finish(c, b, nci, o_ps)


def _proj_strip(c, col0, ncols):
    """Output projection for agT columns [col0, col0+ncols) as 8 units
    (one 128-row output tile each) plus a final batched store on the sync
    ring; the last unit emits the store. y_ps shares the DVE-consumed psA
    pool so proj never contends with the QK^T/exp buffer rotation."""
    nc = c.nc
    st = {}

    def u(ot):
        def f():
            if ot == 0:
                st["yt"] = c.outp.tile([128, CT, 128], BF16, tag="yt",
                                       name="y_t")
            y_t = st["yt"]
            y_ps = c.psA.tile([128, 128], F32, tag="psA", name="y_ps")
            for a in range(CT):
                nc.tensor.matmul(
                    y_ps[:, 0:ncols], c.wp_sb[:, a, 128 * ot:128 * (ot + 1)],
                    c.agT[:, a, col0:col0 + ncols],
                    start=(a == 0), stop=(a == CT - 1))
            nc.vector.tensor_scalar_add(y_t[:, ot, 0:ncols],
                                        y_ps[:, 0:ncols],
                                        c.bmat_sb[:, ot:ot + 1])
            if ot == CT - 1:
                nc.sync.dma_start(
                    c.out_d.ap().rearrange("(o p) t -> p o t",
                                           p=128)[:, :, col0:col0 + ncols],
                    y_t[:, :, 0:ncols])
        return f
    return [u(ot) for ot in range(CT)]


def _build(use_collective=True, reps=1):
    """reps>1 unrolls the whole computation N times inside one NEFF —
    used only for timing (differencing out per-execution overhead)."""
    nc = bacc.Bacc("TRN2", target_bir_lowering=False, debug=False,
                   num_devices=N_CORES if use_collective else 1)
    c = _Ctx()
    c.nc = nc
    c.use_collective = use_collective

    c.xT_d = nc.dram_tensor("xT", [DIM, T], BF16, kind="ExternalInput")
    wqT_d = nc.dram_tensor("wqT", [DIM, 128], BF16, kind="ExternalInput")
    wkT_d = nc.dram_tensor("wkT", [DIM, 128], BF16, kind="ExternalInput")
    wvT_d = nc.dram_tensor("wvT", [DIM, 128], BF16, kind="ExternalInput")
    wpT_d = nc.dram_tensor("wpT", [DIM, DIM], BF16, kind="ExternalInput")
    bmat_d = nc.dram_tensor("bmat", [128, CT], F32, kind="ExternalInput")
    c.out_d = nc.dram_tensor("out", [DIM, TPC], BF16, kind="ExternalOutput")

    with tile.TileContext(nc) as tc:
        with (
            tc.tile_pool(name="const", bufs=1) as const,
            tc.tile_pool(name="xin", bufs=8) as xin,
            tc.tile_pool(name="acts", bufs=1) as acts,
            tc.tile_pool(name="et", bufs=6) as etp,
            tc.tile_pool(name="small", bufs=3) as small,
            tc.tile_pool(name="outp", bufs=4) as outp,
            tc.tile_pool(name="psA", bufs=2, space="PSUM") as psA,
            tc.tile_pool(name="psS", bufs=2, space="PSUM") as psS,
            tc.tile_pool(name="psB", bufs=2, space="PSUM") as psB,
            tc.tile_pool(name="dram", bufs=1, space="DRAM") as dram,
        ):
            c.xin, c.etp, c.small, c.outp = xin, etp, small, outp
            c.psA, c.psS, c.psB = psA, psS, psB

            # ---- constants (k weights first: they gate the critical path) ----
            c.wk_sb = const.tile([128, CT, 128], BF16, name="wk_sb")
            c.wq_sb = const.tile([128, CT, 128], BF16, name="wq_sb")
            c.wv_sb = const.tile([128, CT, 128], BF16, name="wv_sb")
            c.wp_sb = const.tile([128, CT, DIM], BF16, name="wp_sb")
            c.bmat_sb = const.tile([128, CT], F32, name="bmat_sb")
            nc.sync.dma_start(c.wk_sb[:],
                              wkT_d.ap().rearrange("(a p) m -> p a m", p=128))
            nc.sync.dma_start(c.wq_sb[:],
                              wqT_d.ap().rearrange("(a p) m -> p a m", p=128))
            nc.sync.dma_start(c.wv_sb[:],
                              wvT_d.ap().rearrange("(a p) m -> p a m", p=128))
            nc.sync.dma_start(c.bmat_sb[:], bmat_d[:])

            # persistent activations
            c.qT = acts.tile([128, T], BF16, name="qT")
            c.kT = acts.tile([128, T], BF16, name="kT")
            c.v_aug = [acts.tile([128, T // 128, DH + 1], BF16,
                                 name=f"v_aug{h}") for h in range(HPC)]
            c.aT = [acts.tile([DH, T], BF16, name=f"aT{h}")
                    for h in range(HPC)]
            c.agT = acts.tile([128, CT, TPC], BF16, name="agT")

            for h in range(HPC):
                nc.vector.memset(c.v_aug[h][:, :, DH:DH + 1], 1.0)

            # warmup: a few dummy matmuls raise the PE HAM clock gate to
            # 8/8 and a dummy exp preloads the ACT table set, all during
            # the initial x DMA wait.
            warm = acts.tile([128, 512], BF16, name="warm")
            nc.vector.memset(warm[:], 0.0)
            wm_ps = psA.tile([128, 512], F32, tag="psA", name="wm_ps")
            for _w in range(14):
                nc.tensor.matmul(wm_ps[:], warm[:, 0:128], warm[:],
                                 start=(_w == 0), stop=(_w == 13))
            we_t = etp.tile([128, 512], BF16, tag="et", name="we_t")
            nc.scalar.activation(we_t[:], wm_ps[:],
                                 mybir.ActivationFunctionType.Exp)

            c.a2a_in = [dram.tile([N_CORES, HPC, DH, QP], BF16,
                                  name=f"a2a_in{q}") for q in range(NQ)]
            c.a2a_out = [dram.tile([N_CORES, HPC, DH, QP], BF16,
                                   name=f"a2a_out{q}") for q in range(NQ)]

            for _rep in range(reps):
                # chunk 0 compute up front; remaining x loads all queued on
                # the sync ring (bulk, batched) before attention starts;
                # the wp load trails them (first needed ~b0nc2).
                c.xcs = {0: _load_chunk0(c)}
                for f in _k_units(c, 0) + _q_units(c, 0) + _v_units(c, 0):
                    f()
                for tci in range(1, CT):
                    c.xcs[tci] = _load_chunk(c, tci)
                nc.sync.dma_start(
                    c.wp_sb[:], wpT_d.ap().rearrange("(a p) m -> p a m", p=128))

                def sched(*groups):
                    """groups: (units, slot0, per_slot) -> {slot: [fns]}."""
                    iv = {}
                    for units, slot0, per in groups:
                        for i, f in enumerate(units):
                            iv.setdefault(slot0 + i // per, []).append(f)
                    return iv

                # Deadlines: QK^T of m-tile mt reads k of chunk mt//4 (PV
                # reads v one slot later); q of chunk nci by that n-chunk's
                # start; proj strip of quarter s only lands ~6-9us into
                # chunk s+1 (normalize->AllToAll->gather latency), so it
                # fills slots 12-15 there.
                _attn_nchunk(c, 0, 0, interleave=sched(
                    (_k_units(c, 1), 0, 2), (_v_units(c, 1), 2, 2),
                    (_k_units(c, 2), 4, 2), (_v_units(c, 2), 6, 2),
                    (_k_units(c, 3), 8, 2), (_v_units(c, 3), 10, 2),
                    (_q_units(c, 1), 12, 1)))
                _attn_nchunk(c, 0, 1, interleave=sched(
                    (_k_units(c, 4), 0, 1), (_v_units(c, 4), 4, 1),
                    (_q_units(c, 2), 8, 1),
                    (_proj_strip(c, 0, 64), 12, 2)))
                _attn_nchunk(c, 0, 2, interleave=sched(
                    (_k_units(c, 5), 0, 1), (_v_units(c, 5), 4, 1),
                    (_q_units(c, 3), 8, 1),
                    (_proj_strip(c, 64, 64), 12, 2)))
                _attn_nchunk(c, 0, 3, interleave=sched(
                    (_k_units(c, 6), 0, 1), (_q_units(c, 4), 4, 1),
                    (_v_units(c, 6), 8, 1),
                    (_proj_strip(c, 128, 64), 12, 2)))
                _attn_nchunk(c, 1, 0, interleave=sched(
                    (_k_units(c, 7), 0, 1), (_v_units(c, 7), 4, 1),
                    (_q_units(c, 5), 8, 1),
                    (_proj_strip(c, 192, 64), 12, 2)))
                _attn_nchunk(c, 1, 1, interleave=sched(
                    (_q_units(c, 6), 0, 1),
                    (_proj_strip(c, 256, 64), 12, 2)))
                _attn_nchunk(c, 1, 2, interleave=sched(
                    (_q_units(c, 7), 0, 1),
                    (_proj_strip(c, 320, 64), 12, 2)))
                _attn_nchunk(c, 1, 3, interleave=sched(
                    (_proj_strip(c, 384, 64), 12, 2)))
                for f in _proj_strip(c, 448, 64):
                    f()

    nc.compile()
    return nc


def _prep_inputs(x, w_qkv, w_proj, b_proj):
    xf = np.ascontiguousarray(x.reshape(T, DIM).T).astype(ml_dtypes.bfloat16)
    wpT = np.ascontiguousarray(w_proj.T).astype(ml_dtypes.bfloat16)
    bmat = np.ascontiguousarray(b_proj.reshape(CT, 128).T).astype(np.float32)
    in_maps = []
    for c in range(N_CORES):
        r0 = 128 * c
        wqT = np.ascontiguousarray(
            w_qkv[r0:r0 + 128, :].T).astype(ml_dtypes.bfloat16)
        wkT = np.ascontiguousarray(
            w_qkv[DIM + r0:DIM + r0 + 128, :].T).astype(ml_dtypes.bfloat16)
        wvT = np.ascontiguousarray(
            w_qkv[2 * DIM + r0:2 * DIM + r0 + 128, :].T).astype(ml_dtypes.bfloat16)
        in_maps.append({
            "xT": xf, "wqT": wqT, "wkT": wkT, "wvT": wvT,
            "wpT": wpT, "bmat": bmat,
        })
    return in_maps


def _assemble(results):
    out = np.empty((T, DIM), dtype=np.float32)
    for c in range(N_CORES):
        yT = np.asarray(results[c]["out"], dtype=np.float32)  # [DIM, TPC]
        for b in range(B):
            for nci in range(4):
                qi = b * 4 + nci
                t0 = b * N + nci * NCH + c * QP
                col0 = QP * qi
                out[t0:t0 + QP, :] = yT[:, col0:col0 + QP].T
    return out.reshape(B, N, DIM)


def kernel(x, w_qkv, w_proj, b_proj):
    global _cached
    x = np.asarray(x, dtype=np.float32)
    w_qkv = np.asarray(w_qkv, dtype=np.float32)
    w_proj = np.asarray(w_proj, dtype=np.float32)
    b_proj = np.asarray(b_proj, dtype=np.float32)

    if _cached is None:
        _cached = _build()
    nc = _cached

    in_maps = _prep_inputs(x, w_qkv, w_proj, b_proj)
    # the axon terminal occasionally reports a transient device wedge
    # (NRT_EXEC_UNIT_UNRECOVERABLE / mesh desynced) that clears on retry
    last = None
    for attempt in range(3):
        try:
            res = bass_utils.run_bass_kernel_spmd(
                nc, in_maps, core_ids=list(range(N_CORES)))
            return _assemble(res.results)
        except Exception as e:  # noqa: BLE001
            last = e
            import time as _time
            _time.sleep(5 * (attempt + 1))
    raise last


if __name__ == "__main__":
    import jax
    with jax.default_device(jax.devices("cpu")[0]):
        import reference
        inputs = {k: np.asarray(v) for k, v in reference.setup_inputs().items()}
        expected = np.asarray(reference.reference(**inputs))
    actual = kernel(**inputs)
    err = np.linalg.norm(actual - expected) / np.linalg.norm(expected)
    print("Relative error:", err)


# revision 28
# speedup vs baseline: 1.0358x; 1.0358x over previous
"""Distributed multi-head attention kernel for 8 TRN2 NeuronCores.

Problem: x[2,2048,1024] -> qkv proj (w_qkv[3072,1024]) -> 16-head SDPA ->
out proj (w_proj[1024,1024], b_proj[1024]).

Sharding: tensor-parallel over heads. Core c owns heads {2c, 2c+1}:
  - stage 1 (per core): q/k/v for its 2 heads over ALL 4096 tokens,
    transposed score tiles S^T[m,n] per (batch, head), exp on the scalar
    engine (no max-subtraction: scores ~ N(0,1), fp32 exp is safe), PV with
    a trailing ones-column in V so PSUM row DH accumulates the softmax
    denominator, then normalize. Result: aT_h [64 head-dims, 4096 tokens].
  - After EVERY attention n-chunk (512 tokens) a small AllToAll reshards
    that quarter from head-parallel to token-parallel: core c receives the
    full 1024 C-dims for its 64-token piece of the quarter.
  - stage 2 (per core): y^T strips = w_proj @ a + b_proj, emitted as soon
    as both quarters of a 128-col strip have arrived, interleaved into the
    later attention chunks. Only a 64-col strip trails the last AllToAll.

Engine-ring discipline (each engine's queue is strict FIFO):
  - ACT (scalar) ring carries ONLY the exp activations — the softmax pace
    is never blocked behind a DMA trigger.
  - sync (SP) ring: bulk loads (weights, x chunks — one batched DMA per
    chunk), denominator moves, reshard a2a writes + gathers, output stores.
  - gpsimd (Pool) ring: partition broadcasts and the AllToAll collectives.
  - DVE: PSUM->SBUF copies, reciprocal, normalize muls, bias adds.
PSUM pools are split by consumer so the score-tile rotation (ACT-consumed)
is never interleaved with DVE-consumed tiles: a filler allocation between
two s_ps allocations would otherwise couple QK^T emission to exp progress.

Stage-1 compute is interleaved into attention chunks as ~430ns units (2
k/q c-tiles, one v m-tile, or one proj output tile per attention m-tile
slot) so every chunk carries at least the ACT exp pace (~16.6us) without
lumps that head-of-line-block the PE queue: k/v of chunk t+1.. forced
early (QK^T and PV consume them), q deferred to the chunk that first needs
it, proj strips fill the late slots of the chunk after their quarter's
reshard lands.

All TensorE matmuls run in bf16 (fp32 PSUM accumulation); softmax exp is
computed in fp32 on the scalar engine straight out of PSUM.
"""

import numpy as np
import ml_dtypes

import concourse.bass as bass
import concourse.bacc as bacc
import concourse.tile as tile
import concourse.mybir as mybir
from concourse import bass_utils

BF16 = mybir.dt.bfloat16
F32 = mybir.dt.float32

N_CORES = 8
B = 2
N = 2048
DIM = 1024
H = 16
DH = 64
SCALE = DH ** -0.5
HPC = H // N_CORES          # heads per core = 2
T = B * N                   # 4096 global tokens
TPC = T // N_CORES          # 512 tokens per core in stage 2
QP = 64                     # tokens per shard piece per quarter-reshard
CT = DIM // 128             # 8 contraction tiles
TCH = 512                   # token chunk for stage-1 matmul streaming
NCH = 512                   # n (query) chunk in attention
MT = N // 128               # 16 m-tiles per batch
NQ = B * 4                  # 8 quarter-reshards (one per n-chunk)

_cached = None


class _Ctx:
    pass


def _load_chunk0(c):
    """Chunk 0 split per c-tile so the first matmul starts after 1/8 of
    the transfer; c-tiles 4-7 go out on the Pool ring so the two DGE queues
    deliver the chunk in parallel (halves the serial trigger time that
    gates the very first QK^T)."""
    xc = c.xin.tile([128, CT, TCH], BF16, tag="xc", name="xc")
    for a in range(CT):
        eng = c.nc.sync if a < 4 else c.nc.gpsimd
        eng.dma_start(xc[:, a, :], c.xT_d[128 * a:128 * (a + 1), 0:TCH])
    return xc


def _load_chunk(c, tci):
    """One batched DMA per token chunk (single HWDGE trigger)."""
    t0 = tci * TCH
    xc = c.xin.tile([128, CT, TCH], BF16, tag="xc", name="xc")
    c.nc.sync.dma_start(
        xc[:], c.xT_d.ap().rearrange("(a p) t -> p a t", p=128)[:, :, t0:t0 + TCH])
    return xc


def _proj_units(c, w_sb, dst, tci):
    """q/k projection of chunk tci as 4 ~430ns units (2 c-tiles each) so the
    filler can spread across attention m-tile slots without starving the
    ACT-paced exp stream."""
    t0 = tci * TCH
    st = {}

    def u(i):
        def f():
            xc = c.xcs[tci]
            if i == 0:
                st["ps"] = c.psA.tile([128, TCH], F32, tag="psA", name="kq_ps")
            ps = st["ps"]
            for a in (2 * i, 2 * i + 1):
                c.nc.tensor.matmul(ps[:], w_sb[:, a, :], xc[:, a, :],
                                   start=(a == 0), stop=(a == CT - 1))
            if i == 3:
                c.nc.vector.tensor_copy(dst[:, t0:t0 + TCH], ps[:])
        return f
    return [u(i) for i in range(4)]


def _k_units(c, tci):
    return _proj_units(c, c.wk_sb, c.kT, tci)


def _q_units(c, tci):
    return _proj_units(c, c.wq_sb, c.qT, tci)


def _v_units(c, tci):
    """v of chunk tci as 4 units (one 128-token m-tile each)."""
    t0 = tci * TCH

    def u(mt):
        def f():
            xc = c.xcs[tci]
            gmt = (t0 // 128) + mt
            v_ps = c.psA.tile([128, 128], F32, tag="psA", name="v_ps")
            for a in range(CT):
                c.nc.tensor.matmul(
                    v_ps[:], xc[:, a, 128 * mt:128 * (mt + 1)], c.wv_sb[:, a, :],
                    start=(a == 0), stop=(a == CT - 1))
            for h in range(HPC):
                c.nc.vector.tensor_copy(
                    c.v_aug[h][:, gmt, 0:DH], v_ps[:, DH * h:DH * (h + 1)])
        return f
    return [u(mt) for mt in range(TCH // 128)]


def _attn_qk_exp(c, b, nci, mt):
    nc = c.nc
    n0 = b * N + nci * NCH
    m0 = b * N + 128 * mt
    s_ps = c.psS.tile([128, HPC * NCH], F32, tag="psS", name="s_ps")
    e_t = c.etp.tile([128, HPC * NCH], BF16, tag="et", name="e_t")
    for h in range(HPC):
        nc.tensor.matmul(
            s_ps[:, NCH * h:NCH * (h + 1)],
            c.kT[DH * h:DH * (h + 1), m0:m0 + 128],
            c.qT[DH * h:DH * (h + 1), n0:n0 + NCH],
            start=True, stop=True,
            tile_position=(DH * h, 0))
    nc.scalar.activation(e_t[:], s_ps[:],
                         mybir.ActivationFunctionType.Exp, scale=SCALE)
    return e_t


def _attn_pv(c, mt, gmt, e_t, o_ps):
    nc = c.nc
    for h in range(HPC):
        nc.tensor.matmul(
            o_ps[h][:], c.v_aug[h][:, gmt, :],
            e_t[:, NCH * h:NCH * (h + 1)],
            start=(mt == 0), stop=(mt == MT - 1))


def _attn_finish(c, b, nci, o_ps):
    """Normalize + quarter-reshard, pipelined per head to minimize the
    serial latency from the last PV to the AllToAll gather (this chain
    gates both the chunk-boundary o_ps buffer turnaround and the kernel
    tail). Row DH of o_ps is the softmax denominator: DVE reciprocal, a
    tiny SP-ring DMA moves it to partition 0, gpsimd broadcasts, DVE
    scales, SP-ring DMA stages the a2a slabs."""
    nc = c.nc
    qi = b * 4 + nci
    n0 = b * N + nci * NCH
    den = c.small.tile([DH + 1, HPC * NCH], F32, tag="den", name="den")
    rden = c.small.tile([1, HPC * NCH], F32, tag="rden", name="rden")
    rb = c.small.tile([DH, HPC * NCH], F32, tag="rb", name="rb")
    for h in range(HPC):
        nc.vector.reciprocal(den[DH:DH + 1, NCH * h:NCH * (h + 1)],
                             o_ps[h][DH:DH + 1, :])
        nc.sync.dma_start(rden[:, NCH * h:NCH * (h + 1)],
                          den[DH:DH + 1, NCH * h:NCH * (h + 1)])
    for h in range(HPC):
        nc.gpsimd.partition_broadcast(rb[:, NCH * h:NCH * (h + 1)],
                                      rden[:, NCH * h:NCH * (h + 1)])
        nc.vector.tensor_mul(c.aT[h][:, n0:n0 + NCH],
                             o_ps[h][0:DH, :],
                             rb[:, NCH * h:NCH * (h + 1)])
        nc.sync.dma_start(
            c.a2a_in[qi][:, h, :, :].rearrange("j d t -> d j t"),
            c.aT[h][:, n0:n0 + NCH])
    if c.use_collective:
        nc.gpsimd.collective_compute(
            "AllToAll", mybir.AluOpType.bypass,
            replica_groups=[list(range(N_CORES))],
            ins=[c.a2a_in[qi].opt()], outs=[c.a2a_out[qi].opt()])
    else:
        nc.gpsimd.dma_start(c.a2a_out[qi][:], c.a2a_in[qi][:])
    for wh in range(2):
        nc.sync.dma_start(
            c.agT[:, 4 * wh:4 * wh + 4, QP * qi:QP * qi + QP],
            c.a2a_out[qi][4 * wh:4 * wh + 4].rearrange("w h d t -> (h d) w t"))


def _attn_nchunk(c, b, nci, interleave=None):
    """One attention n-chunk; interleave maps m-tile slot -> list of filler
    callbacks emitted after that m-tile's QK^T/exp + the pipelined PV, to
    fill TensorEngine slack at sub-exp granularity. Ends with this
    quarter's normalize + reshard."""
    o_ps = [c.psB.tile([DH + 1, NCH], F32, tag="psB", name=f"o_ps{h}")
            for h in range(HPC)]
    # software-pipelined: PV of m-tile mt-1 is emitted after QK^T/exp of
    # m-tile mt, so the next QK^T never queues behind an exp-blocked PV.
    prev = None
    for mt in range(MT):
        e_t = _attn_qk_exp(c, b, nci, mt)
        # fillers sit BETWEEN this QK^T and the exp-gated PV of the
        # previous m-tile, so the PE chews filler during the exp wait
        # instead of idling at the PV head-of-line.
        if interleave and mt in interleave:
            for fn in interleave[mt]:
                fn()
        if prev is not None:
            _attn_pv(c, prev[0], b * MT + prev[0], prev[1], o_ps)
        prev = (mt, e_t)
    _attn_pv(c, prev[0], b * MT + prev[0], prev[1], o_ps)
    _attn_finish(c, b, nci, o_ps)


def _proj_strip(c, col0, ncols):
    """Output projection for agT columns [col0, col0+ncols) as 8 units
    (one 128-row output tile each) plus a final batched store on the sync
    ring; the last unit emits the store. y_ps shares the DVE-consumed psA
    pool so proj never contends with the QK^T/exp buffer rotation."""
    nc = c.nc
    st = {}

    def u(ot):
        def f():
            if ot == 0:
                st["yt"] = c.outp.tile([128, CT, 128], BF16, tag="yt",
                                       name="y_t")
            y_t = st["yt"]
            y_ps = c.psA.tile([128, 128], F32, tag="psA", name="y_ps")
            for a in range(CT):
                nc.tensor.matmul(
                    y_ps[:, 0:ncols], c.wp_sb[:, a, 128 * ot:128 * (ot + 1)],
                    c.agT[:, a, col0:col0 + ncols],
                    start=(a == 0), stop=(a == CT - 1))
            nc.vector.tensor_scalar_add(y_t[:, ot, 0:ncols],
                                        y_ps[:, 0:ncols],
                                        c.bmat_sb[:, ot:ot + 1])
            if ot == CT - 1:
                nc.sync.dma_start(
                    c.out_d.ap().rearrange("(o p) t -> p o t",
                                           p=128)[:, :, col0:col0 + ncols],
                    y_t[:, :, 0:ncols])
        return f
    return [u(ot) for ot in range(CT)]


def _build(use_collective=True, reps=1):
    """reps>1 unrolls the whole computation N times inside one NEFF —
    used only for timing (differencing out per-execution overhead)."""
    nc = bacc.Bacc("TRN2", target_bir_lowering=False, debug=False,
                   num_devices=N_CORES if use_collective else 1)
    c = _Ctx()
    c.nc = nc
    c.use_collective = use_collective

    c.xT_d = nc.dram_tensor("xT", [DIM, T], BF16, kind="ExternalInput")
    wqT_d = nc.dram_tensor("wqT", [DIM, 128], BF16, kind="ExternalInput")
    wkT_d = nc.dram_tensor("wkT", [DIM, 128], BF16, kind="ExternalInput")
    wvT_d = nc.dram_tensor("wvT", [DIM, 128], BF16, kind="ExternalInput")
    wpT_d = nc.dram_tensor("wpT", [DIM, DIM], BF16, kind="ExternalInput")
    bmat_d = nc.dram_tensor("bmat", [128, CT], F32, kind="ExternalInput")
    c.out_d = nc.dram_tensor("out", [DIM, TPC], BF16, kind="ExternalOutput")

    with tile.TileContext(nc) as tc:
        with (
            tc.tile_pool(name="const", bufs=1) as const,
            tc.tile_pool(name="xin", bufs=8) as xin,
            tc.tile_pool(name="acts", bufs=1) as acts,
            tc.tile_pool(name="et", bufs=6) as etp,
            tc.tile_pool(name="small", bufs=3) as small,
            tc.tile_pool(name="outp", bufs=4) as outp,
            tc.tile_pool(name="psA", bufs=2, space="PSUM") as psA,
            tc.tile_pool(name="psS", bufs=2, space="PSUM") as psS,
            tc.tile_pool(name="psB", bufs=2, space="PSUM") as psB,
            tc.tile_pool(name="dram", bufs=1, space="DRAM") as dram,
        ):
            c.xin, c.etp, c.small, c.outp = xin, etp, small, outp
            c.psA, c.psS, c.psB = psA, psS, psB

            # ---- constants (k weights first: they gate the critical path) ----
            c.wk_sb = const.tile([128, CT, 128], BF16, name="wk_sb")
            c.wq_sb = const.tile([128, CT, 128], BF16, name="wq_sb")
            c.wv_sb = const.tile([128, CT, 128], BF16, name="wv_sb")
            c.wp_sb = const.tile([128, CT, DIM], BF16, name="wp_sb")
            c.bmat_sb = const.tile([128, CT], F32, name="bmat_sb")
            nc.sync.dma_start(c.wk_sb[:],
                              wkT_d.ap().rearrange("(a p) m -> p a m", p=128))
            nc.sync.dma_start(c.wq_sb[:],
                              wqT_d.ap().rearrange("(a p) m -> p a m", p=128))
            nc.sync.dma_start(c.wv_sb[:],
                              wvT_d.ap().rearrange("(a p) m -> p a m", p=128))
            nc.sync.dma_start(c.bmat_sb[:], bmat_d[:])

            # persistent activations
            c.qT = acts.tile([128, T], BF16, name="qT")
            c.kT = acts.tile([128, T], BF16, name="kT")
            c.v_aug = [acts.tile([128, T // 128, DH + 1], BF16,
                                 name=f"v_aug{h}") for h in range(HPC)]
            c.aT = [acts.tile([DH, T], BF16, name=f"aT{h}")
                    for h in range(HPC)]
            c.agT = acts.tile([128, CT, TPC], BF16, name="agT")

            for h in range(HPC):
                nc.vector.memset(c.v_aug[h][:, :, DH:DH + 1], 1.0)

            # warmup: a few dummy matmuls raise the PE HAM clock gate to
            # 8/8 and a dummy exp preloads the ACT table set, all during
            # the initial x DMA wait.
            warm = acts.tile([128, 512], BF16, name="warm")
            nc.vector.memset(warm[:], 0.0)
            wm_ps = psA.tile([128, 512], F32, tag="psA", name="wm_ps")
            for _w in range(14):
                nc.tensor.matmul(wm_ps[:], warm[:, 0:128], warm[:],
                                 start=(_w == 0), stop=(_w == 13))
            we_t = etp.tile([128, 512], BF16, tag="et", name="we_t")
            nc.scalar.activation(we_t[:], wm_ps[:],
                                 mybir.ActivationFunctionType.Exp)

            c.a2a_in = [dram.tile([N_CORES, HPC, DH, QP], BF16,
                                  name=f"a2a_in{q}") for q in range(NQ)]
            c.a2a_out = [dram.tile([N_CORES, HPC, DH, QP], BF16,
                                   name=f"a2a_out{q}") for q in range(NQ)]

            for _rep in range(reps):
                # chunk 0 compute up front; remaining x loads all queued on
                # the sync ring (bulk, batched) before attention starts;
                # the wp load trails them (first needed ~b0nc2).
                c.xcs = {0: _load_chunk0(c)}
                for f in _k_units(c, 0) + _q_units(c, 0) + _v_units(c, 0):
                    f()
                for tci in range(1, CT):
                    c.xcs[tci] = _load_chunk(c, tci)
                nc.sync.dma_start(
                    c.wp_sb[:], wpT_d.ap().rearrange("(a p) m -> p a m", p=128))

                def sched(*groups):
                    """groups: (units, slot0, per_slot) -> {slot: [fns]}."""
                    iv = {}
                    for units, slot0, per in groups:
                        for i, f in enumerate(units):
                            iv.setdefault(slot0 + i // per, []).append(f)
                    return iv

                # Deadlines: QK^T of m-tile mt reads k of chunk mt//4 (PV
                # reads v one slot later); q of chunk nci by that n-chunk's
                # start; proj strip of quarter s only lands ~6-9us into
                # chunk s+1 (normalize->AllToAll->gather latency), so it
                # fills slots 12-15 there.
                _attn_nchunk(c, 0, 0, interleave=sched(
                    (_k_units(c, 1), 0, 2), (_v_units(c, 1), 2, 2),
                    (_k_units(c, 2), 4, 2), (_v_units(c, 2), 6, 2),
                    (_k_units(c, 3), 8, 2), (_v_units(c, 3), 10, 2),
                    (_q_units(c, 1), 12, 1)))
                _attn_nchunk(c, 0, 1, interleave=sched(
                    (_k_units(c, 4), 0, 1), (_v_units(c, 4), 4, 1),
                    (_q_units(c, 2), 8, 1),
                    (_proj_strip(c, 0, 64), 12, 2)))
                _attn_nchunk(c, 0, 2, interleave=sched(
                    (_k_units(c, 5), 0, 1), (_v_units(c, 5), 4, 1),
                    (_q_units(c, 3), 8, 1),
                    (_proj_strip(c, 64, 64), 12, 2)))
                _attn_nchunk(c, 0, 3, interleave=sched(
                    (_k_units(c, 6), 0, 1), (_q_units(c, 4), 4, 1),
                    (_v_units(c, 6), 8, 1),
                    (_proj_strip(c, 128, 64), 12, 2)))
                _attn_nchunk(c, 1, 0, interleave=sched(
                    (_k_units(c, 7), 0, 1), (_v_units(c, 7), 4, 1),
                    (_q_units(c, 5), 8, 1),
                    (_proj_strip(c, 192, 64), 12, 2)))
                _attn_nchunk(c, 1, 1, interleave=sched(
                    (_q_units(c, 6), 0, 1),
                    (_proj_strip(c, 256, 64), 12, 2)))
                _attn_nchunk(c, 1, 2, interleave=sched(
                    (_q_units(c, 7), 0, 1),
                    (_proj_strip(c, 320, 64), 12, 2)))
                _attn_nchunk(c, 1, 3, interleave=sched(
                    (_proj_strip(c, 384, 64), 12, 2)))
                for f in _proj_strip(c, 448, 64):
                    f()

    nc.compile()
    return nc


def _prep_inputs(x, w_qkv, w_proj, b_proj):
    xf = np.ascontiguousarray(x.reshape(T, DIM).T).astype(ml_dtypes.bfloat16)
    wpT = np.ascontiguousarray(w_proj.T).astype(ml_dtypes.bfloat16)
    bmat = np.ascontiguousarray(b_proj.reshape(CT, 128).T).astype(np.float32)
    in_maps = []
    for c in range(N_CORES):
        r0 = 128 * c
        wqT = np.ascontiguousarray(
            w_qkv[r0:r0 + 128, :].T).astype(ml_dtypes.bfloat16)
        wkT = np.ascontiguousarray(
            w_qkv[DIM + r0:DIM + r0 + 128, :].T).astype(ml_dtypes.bfloat16)
        wvT = np.ascontiguousarray(
            w_qkv[2 * DIM + r0:2 * DIM + r0 + 128, :].T).astype(ml_dtypes.bfloat16)
        in_maps.append({
            "xT": xf, "wqT": wqT, "wkT": wkT, "wvT": wvT,
            "wpT": wpT, "bmat": bmat,
        })
    return in_maps


def _assemble(results):
    out = np.empty((T, DIM), dtype=np.float32)
    for c in range(N_CORES):
        yT = np.asarray(results[c]["out"], dtype=np.float32)  # [DIM, TPC]
        for b in range(B):
            for nci in range(4):
                qi = b * 4 + nci
                t0 = b * N + nci * NCH + c * QP
                col0 = QP * qi
                out[t0:t0 + QP, :] = yT[:, col0:col0 + QP].T
    return out.reshape(B, N, DIM)


def kernel(x, w_qkv, w_proj, b_proj):
    global _cached
    x = np.asarray(x, dtype=np.float32)
    w_qkv = np.asarray(w_qkv, dtype=np.float32)
    w_proj = np.asarray(w_proj, dtype=np.float32)
    b_proj = np.asarray(b_proj, dtype=np.float32)

    if _cached is None:
        _cached = _build()
    nc = _cached

    in_maps = _prep_inputs(x, w_qkv, w_proj, b_proj)
    # the axon terminal occasionally reports a transient device wedge
    # (NRT_EXEC_UNIT_UNRECOVERABLE / mesh desynced) that clears on retry
    last = None
    for attempt in range(3):
        try:
            res = bass_utils.run_bass_kernel_spmd(
                nc, in_maps, core_ids=list(range(N_CORES)))
            return _assemble(res.results)
        except Exception as e:  # noqa: BLE001
            last = e
            import time as _time
            _time.sleep(5 * (attempt + 1))
    raise last


if __name__ == "__main__":
    import jax
    with jax.default_device(jax.devices("cpu")[0]):
        import reference
        inputs = {k: np.asarray(v) for k, v in reference.setup_inputs().items()}
        expected = np.asarray(reference.reference(**inputs))
    actual = kernel(**inputs)
    err = np.linalg.norm(actual - expected) / np.linalg.norm(expected)
    print("Relative error:", err)
